# revision 1
# baseline (speedup 1.0000x reference)
"""Causal MHA attention-out kernel for TRN2, head-sharded across 8 NeuronCores.

Reference computation (fp32):
    scores = (q @ k^T) / sqrt(64), causal mask, softmax
    z      = pattern @ v
    out    = sum_h z_h @ W_O[h] + b_O          (residual passed through)

Sharding: 16 heads -> 8 cores x 2 adjacent heads. Each core computes a
partial out (its 2 heads' contribution, both batches); host sums partials.

Per-core layout (per batch b):
  kT/qT  [128, 2048]   d-major (head0 -> partitions 0-63, head1 -> 64-127),
                       loaded via bf16 xbar DMA transpose (dma_start_transpose).
  Pipeline unit = one k-block (128 k positions) for a 512-wide q chunk,
  BOTH heads sharing one [128,1024] fp32 PSUM score tile (h0 cols 0-511,
  h1 cols 512-1023). The ACT engine is the softmax-exp bottleneck (~78us
  of exp per core is irreducible: no other engine has activation tables),
  so ACT does exp ONLY -- one instruction per off-diagonal unit, two per
  diagonal unit. Causal masking is folded into the scores: a constant
  -16384 strict-upper-triangle is matmul'd (ident @ triu) into each
  diagonal 128x128 score block before QK accumulates onto it, so exp
  yields exact zeros there and PV depends on nothing but exp. Normalize
  runs on DVE (reciprocal + mul) with a Pool partition_broadcast.
  Projection goes PSUM -> bf16 osb staging (copies split 50/50 between
  DVE and ACT: HW calibration measured DVE work carrying the highest
  critical-path coefficient while ACT had slack) -> one consolidated
  SWDGE out-DMA per (b,qc) issued from Pool.

  Engine-queue roles: SP carries ONLY input loads (4 whole-tensor xbar
  transposes + v loads), so its program runs a full For_i iteration ahead
  of compute -- true cross-iteration prefetch; Pool owns out-DMAs.

  Emission is one globally software-pipelined stream over all (b,qc,kb)
  units: QK(u) leads PV(u) by PV_LAG units so the in-order PE sequencer
  never blocks on the exp chain (PE p-state: a continuously-busy PE runs
  2x faster than one that keeps stalling); chunk tails never stall the
  next chunk's QKs. Projection ops are deferred and injected one per unit,
  no earlier than INJECT_DELAY units after their normalize, so a
  not-yet-ready zsb never head-of-line-blocks the PE sequencer.
  PSUM: 3x2-bank score slots (shared with projection outputs) + 2x1-bank
  z accumulators = 8 banks exactly.
"""

import numpy as np

import concourse.bass as bass
import concourse.mybir as mybir
from concourse import bacc
import concourse.tile as tile
from concourse.bass_utils import run_bass_kernel_spmd

B = 2
S = 2048
D_MODEL = 1024
N_HEADS = 16
D_HEAD = 64
N_CORES = 8
HPC = 2  # heads per core
CW = HPC * D_HEAD  # 128 columns of q/k/v per core
NKB = S // 128  # 16 k-blocks
NQC = S // 512  # 4 q-chunks
INV_SCALE = 1.0 / 8.0  # 1/sqrt(64)

F32 = mybir.dt.float32
MMDT = mybir.dt.bfloat16  # matmul operand dtype: guaranteed 1 cyc/row on PE

import os
PV_LAG = int(os.environ.get("PV_LAG", "4"))  # units QK leads PV
INJECT_DELAY = int(os.environ.get("INJECT_DELAY", "3"))  # units QK runs ahead of PV

_CACHE = {}


def _build_bass(reps=None, py_reps=1):
    nc = bacc.Bacc("TRN2", target_bir_lowering=False)

    q_d = nc.dram_tensor("q", [B, S, CW], MMDT, kind="ExternalInput")
    k_d = nc.dram_tensor("k", [B, S, CW], MMDT, kind="ExternalInput")
    v_d = nc.dram_tensor("v", [B, S, CW], MMDT, kind="ExternalInput")
    wo_d = nc.dram_tensor("wo", [CW, D_MODEL], MMDT, kind="ExternalInput")
    out_d = nc.dram_tensor("out", [B, S, D_MODEL], MMDT, kind="ExternalOutput")

    with tile.TileContext(nc) as tc:
        with (
            tc.tile_pool(name="const", bufs=1) as const_pool,
            tc.tile_pool(name="big", bufs=4) as big_pool,
            tc.tile_pool(name="stage", bufs=4) as stage_pool,
            tc.tile_pool(name="pat", bufs=8) as pat_pool,
            tc.tile_pool(name="osb", bufs=4) as osb_pool,
            tc.tile_pool(name="psc", bufs=3, space="PSUM") as psc_pool,
            tc.tile_pool(name="pz", bufs=2, space="PSUM") as pz_pool,
        ):
            ones16 = const_pool.tile([128, NKB], F32)
            nc.gpsimd.memset(ones16, 1.0)
            from concourse.masks import make_identity
            ident_f = const_pool.tile([128, 128], F32)
            make_identity(nc, ident_f)
            ident = const_pool.tile([128, 128], MMDT)
            nc.vector.tensor_copy(ident, ident_f)
            # triu_neg[p, j] = -16384 where p > j else 0 (strict upper tri in
            # [k, q] coords = the causally-masked half of a diagonal block)
            triu_f = const_pool.tile([128, 128], F32)
            nc.gpsimd.memset(triu_f, -16384.0)
            nc.gpsimd.affine_select(
                out=triu_f,
                in_=triu_f,
                compare_op=mybir.AluOpType.is_ge,
                fill=0.0,
                base=-1,
                pattern=[[-1, 128]],
                channel_multiplier=1,
            )
            triu_neg = const_pool.tile([128, 128], MMDT)
            nc.vector.tensor_copy(triu_neg, triu_f)
            wo_sb = const_pool.tile([CW, D_MODEL], MMDT)
            nc.sync.dma_start(wo_sb, wo_d[:, :])

            import contextlib

            loop_cm = (
                tc.For_i(
                    0,
                    reps,
                    1,
                    hint_engines=(
                        mybir.EngineType.PE,
                        mybir.EngineType.DVE,
                        mybir.EngineType.Activation,
                        mybir.EngineType.Pool,
                        mybir.EngineType.SP,
                    ),
                    staggered_reset=True,
                )
                if reps
                else contextlib.nullcontext()
            )
            with loop_cm:
                for _pr in range(py_reps):
                    _emit_body(nc, tc, locals())
    nc.compile()
    return nc


def _emit_body(nc, tc, env):
    (q_d, k_d, v_d, wo_d, out_d) = (
        env["q_d"], env["k_d"], env["v_d"], env["wo_d"], env["out_d"]
    )
    (const_pool, big_pool, stage_pool, pat_pool, psc_pool, pz_pool) = (
        env["const_pool"], env["big_pool"], env["stage_pool"], env["pat_pool"],
        env["psc_pool"], env["pz_pool"]
    )
    osb_pool = env["osb_pool"]
    ones16, wo_sb = env["ones16"], env["wo_sb"]
    ident, triu_neg = env["ident"], env["triu_neg"]
    # calibration knobs: duplicate one engine's instructions to measure the
    # HW marginal cost of that engine (timing builds only)
    dup_exp = int(os.environ.get("DUP_EXP", "1"))
    dup_mm = int(os.environ.get("DUP_MM", "1"))
    dup_dve = int(os.environ.get("DUP_DVE", "1"))
    dup_tp = int(os.environ.get("DUP_TPOSE", "1"))

    kTs, qTs, vbigs = [], [], []
    for b in range(B):
        kT = big_pool.tile([128, S], MMDT, tag="kT", name=f"kT{b}")
        qT = big_pool.tile([128, S], MMDT, tag="qT", name=f"qT{b}")
        # v packed per k-block as [v_h0 | ones | v_h1 | ones] (130 cols)
        vbig = big_pool.tile([128, NKB * 130], MMDT, tag="vb", name=f"vb{b}")
        kTs.append(kT); qTs.append(qT); vbigs.append(vbig)
        # bf16 enables the xbar DMA transpose: one transposing DMA
        # per tensor replaces PE transposes + DVE copies entirely.
        # SP carries ONLY input loads: issuing a transpose costs ~0.7us of
        # sequencer time, and with nothing queued behind them SP's program
        # runs a full For_i iteration ahead -- true cross-iteration prefetch.
        # Out-DMAs go to Pool's SWDGE queue instead.
        for src_, dstT in ((k_d, kT), (q_d, qT)):
            for _d in range(dup_tp):
                nc.sync.dma_start_transpose(dstT, src_[b])
        v3 = vbig.rearrange("p (t c) -> p t c", c=130)
        nc.sync.dma_start(
            v3[:, :, 0:64],
            v_d[b].rearrange("(t p) c -> p t c", p=128)[:, :, 0:64],
        )
        nc.sync.dma_start(
            v3[:, :, 65:129],
            v_d[b].rearrange("(t p) c -> p t c", p=128)[:, :, 64:128],
        )
        nc.vector.tensor_copy(v3[:, :, 64], ones16)
        nc.vector.tensor_copy(v3[:, :, 129], ones16)

    # Deferred projection ops of completed (b,qc) chunks: injected between
    # units of later chunks so the PE/psc ring never drains.
    pending = []  # list of (earliest_index, closure) emitting one proj op
    cur_idx = [0]

    def emit_some_pending(k):
        n = 0
        while pending and n < k and pending[0][0] <= cur_idx[0]:
            pending.pop(0)[1]()
            n += 1

    def make_proj(b, qc, zsb, tail=False):
        # one [128, 4*1024] staging tile per (b,qc); a single consolidated
        # out-DMA on Pool's SWDGE queue (994ns fixed cost per DMA, so batch)
        osb = osb_pool.tile([128, 4 * D_MODEL], MMDT, tag="osb",
                            name=f"osb{b}_{qc}")

        def one_op(qb):
            def emit():
                op = psc_pool.tile([128, 1024], F32, tag="sc",
                                   name=f"op{b}_{qc}_{qb}")
                for mch in range(2):
                    nc.tensor.matmul(
                        op[:, mch * 512 : (mch + 1) * 512],
                        lhsT=zsb[:, qb * 128 : (qb + 1) * 128],
                        rhs=wo_sb[:, mch * 512 : (mch + 1) * 512],
                        start=True,
                        stop=True,
                    )
                dst = osb[:, qb * 1024 : (qb + 1) * 1024]
                for _d in range(dup_dve):
                    # split copies DVE/ACT: per HW calibration DVE work carries
                    # the highest critical-path coefficient, ACT has slack
                    if qb % 2 == 1:
                        nc.scalar.copy(dst, op)
                    else:
                        nc.vector.tensor_copy(dst, op)
                if qb == 3:
                    ddst = out_d[b, qc * 512 : (qc + 1) * 512, :].rearrange(
                        "(qb p) m -> p qb m", p=128
                    )
                    src = osb.rearrange("p (qb m) -> p qb m", m=D_MODEL)
                    nc.gpsimd.dma_start(ddst, src)
            return emit

        return [one_op(qb) for qb in range(4)]

    # One globally software-pipelined stream over every (b, qc, kb) unit:
    # batches interleave at chunk granularity, QK leads PV by PV_LAG units,
    # and chunk tails (PV drain / normalize / projection) never stall the
    # in-order PE sequencer because the next chunk's QKs are emitted first.
    stream = []
    for qc in range(NQC):
        for b in range(B):
            for kb in range(4 * qc + 4):
                stream.append((b, qc, kb))

    zaccs = {}  # (b, qc) -> [h0, h1] PSUM accumulators
    pats = {}   # (b, qc, kb) -> pattern tile

    def emit_qk_exp(u):
        b, qc, kb = u
        kT, qT = kTs[b], qTs[b]
        dd = kb - 4 * qc
        s = 128 * dd if dd > 0 else 0
        sc = psc_pool.tile([128, 1024], F32, tag="sc", name=f"sc{b}_{qc}_{kb}")
        for h in range(HPC):
            if dd >= 0:
                nc.tensor.matmul(
                    sc[:, 512 * h + s : 512 * h + s + 128],
                    lhsT=ident,
                    rhs=triu_neg,
                    start=True,
                    stop=False,
                )
                for _d in range(dup_mm):
                    nc.tensor.matmul(
                        sc[:, 512 * h + s : 512 * h + s + 128],
                        lhsT=kT[64 * h : 64 * h + 64, kb * 128 : (kb + 1) * 128],
                        rhs=qT[64 * h : 64 * h + 64,
                               qc * 512 + s : qc * 512 + s + 128],
                        start=False,
                        stop=True,
                    )
                if s + 128 < 512:
                    for _d in range(dup_mm):
                        nc.tensor.matmul(
                            sc[:, 512 * h + s + 128 : 512 * h + 512],
                            lhsT=kT[64 * h : 64 * h + 64, kb * 128 : (kb + 1) * 128],
                            rhs=qT[64 * h : 64 * h + 64,
                                   qc * 512 + s + 128 : (qc + 1) * 512],
                            start=True,
                            stop=True,
                        )
            else:
                for _d in range(dup_mm):
                    nc.tensor.matmul(
                        sc[:, 512 * h + s : 512 * h + 512],
                        lhsT=kT[64 * h : 64 * h + 64, kb * 128 : (kb + 1) * 128],
                        rhs=qT[64 * h : 64 * h + 64, qc * 512 + s : (qc + 1) * 512],
                        start=True,
                        stop=True,
                    )
        pt = pat_pool.tile([128, 1024], MMDT, tag="pat", name=f"pat{b}_{qc}_{kb}")
        pats[u] = pt
        # exp (ACT reads PSUM, scale=1/8 folded in); one instruction for
        # off-diagonal units, two for diagonal (skip the masked-out cols)
        if dd <= 0:
            eranges = [(0, 1024)]
        else:
            eranges = [(s, 512), (512 + s, 1024)]
        for e0, e1 in eranges:
            for _d in range(dup_exp):
                nc.scalar.activation(
                    pt[:, e0:e1],
                    sc[:, e0:e1],
                    mybir.ActivationFunctionType.Exp,
                    scale=INV_SCALE,
                )

    def emit_pv(u):
        b, qc, kb = u
        last_kb = 4 * qc + 3
        if kb == 0:
            zaccs[(b, qc)] = [
                pz_pool.tile([65, 512], F32, tag="z", name=f"zacc{b}_{qc}_{h}")
                for h in range(HPC)
            ]
        zacc = zaccs[(b, qc)]
        dd = kb - 4 * qc
        s = 128 * dd if dd > 0 else 0
        for h in range(HPC):
            for _d in range(dup_mm):
                nc.tensor.matmul(
                    zacc[h][:, s:512],
                    lhsT=vbigs[b][:, kb * 130 + 65 * h : kb * 130 + 65 * h + 65],
                    rhs=pats[u][:, 512 * h + s : 512 * h + 512],
                    start=(kb == 0 and _d == 0),
                    stop=(kb == last_kb and _d == dup_mm - 1),
                )
        if kb == last_kb:
            emit_normalize(b, qc)

    def emit_normalize(b, qc):
        # normalize: z = z / denom  (DVE reciprocal + mul, Pool broadcast)
        zacc = zaccs[(b, qc)]
        zsb = stage_pool.tile([128, 512], MMDT, tag="zsb", name=f"zsb{b}_{qc}")
        r_sbs, rbs = [], []
        for h in range(HPC):
            r_sb = stage_pool.tile([1, 512], F32, tag="r")
            nc.vector.reciprocal(r_sb, zacc[h][64:65, :])
            r_sbs.append(r_sb)
        for h in range(HPC):
            rb = stage_pool.tile([64, 512], F32, tag="rb")
            nc.gpsimd.partition_broadcast(rb, r_sbs[h])
            rbs.append(rb)
        for h in range(HPC):
            for _d in range(dup_dve):
                nc.vector.tensor_mul(
                    zsb[64 * h : 64 * h + 64, :],
                    zacc[h][0:64, :],
                    rbs[h],
                )
        ops = make_proj(b, qc, zsb,
                        tail=(b == B - 1 and qc == NQC - 1
                              and os.environ.get("TAIL_ACT", "1") == "1"))
        pending.extend((cur_idx[0] + INJECT_DELAY, op) for op in ops)

    for i, u in enumerate(stream):
        cur_idx[0] = i
        emit_qk_exp(u)
        emit_some_pending(1)
        if i >= PV_LAG:
            emit_pv(stream[i - PV_LAG])
    for u in stream[-PV_LAG:]:
        emit_some_pending(1)
        emit_pv(u)

    # drain whatever projection ops remain at the end of the iteration
    cur_idx[0] = float("inf")
    emit_some_pending(len(pending))


def make_in_maps(q, k, v, W_O):
    import ml_dtypes

    bf16 = ml_dtypes.bfloat16
    q = np.asarray(q, dtype=np.float32).astype(bf16)
    k = np.asarray(k, dtype=np.float32).astype(bf16)
    v = np.asarray(v, dtype=np.float32).astype(bf16)
    W_O = np.asarray(W_O, dtype=np.float32).astype(bf16)
    in_maps = []
    for c in range(N_CORES):
        cols = slice(c * CW, (c + 1) * CW)
        in_maps.append(
            {
                "q": np.ascontiguousarray(q[:, :, cols]),
                "k": np.ascontiguousarray(k[:, :, cols]),
                "v": np.ascontiguousarray(v[:, :, cols]),
                "wo": np.ascontiguousarray(
                    W_O[c * HPC : (c + 1) * HPC].reshape(CW, D_MODEL)
                ),
            }
        )
    return in_maps


def get_nc():
    if "nc" not in _CACHE:
        _CACHE["nc"] = _build_bass()
    return _CACHE["nc"]


def kernel(q, k, v, residual, W_O, b_O):
    nc = get_nc()
    in_maps = make_in_maps(q, k, v, W_O)
    res = run_bass_kernel_spmd(nc, in_maps, core_ids=list(range(N_CORES)))
    out = res.results[0]["out"].astype(np.float64)
    for r in res.results[1:]:
        out += r["out"].astype(np.float64)
    out = (out + np.asarray(b_O, dtype=np.float64)[None, None, :]).astype(np.float32)
    return out, np.asarray(residual)

